# revision 12
# baseline (speedup 1.0000x reference)
"""Bass/Tile TRN2 kernel for nn_Bonv_89369679495333 (SAGE-conv + dense_diff_pool).

Strategy (8 NeuronCores, row-sharded over the N=8192 nodes, 1024 rows/core):
  phase 1: t = [x_hi | x_lo | 1]^T @ A_shard   (f32r split matmuls, exact to ~2^-26)
           -> AllReduce [5, 8192]  -> t_x = Atx rows, deg = colsum row
  prep:    agg = t_x / max(deg,1); logits = [agg;nodes;1]^T W2cat ; s = softmax
           x1 = [agg;nodes;1]^T W1cat   (conv1 output)
           s, x1 written to DRAM; each core gathers its local 1024 rows via a
           partition_id-offset DMA (the program is SPMD-identical on all cores).
  phase 2: zT_c = s_local^T @ A_shard  (two f32r passes: s_hi 13-bit + remainder)
           z chunks PE-transposed; adj_pool partial = z^T s accumulated in PSUM
           G = s_loc^T s_loc, x_out = s_loc^T x1_loc, ent partial
           -> AllReduce [128, 259]
  epilogue (replicated): E = rowmax-equality, argmax, x3 = SAGE(E), link_loss via
           ||A - s s^T||^2 = nnz(A) - 2 tr(adj_pool) + ||s^T s||^2.
"""

import numpy as np

import concourse.bass as bass
import concourse.bacc as bacc
import concourse.tile as tile
import concourse.mybir as mybir
import concourse.bass_utils as bass_utils

F32 = mybir.dt.float32
F32R = mybir.dt.float32r
BF16 = mybir.dt.bfloat16
I32 = mybir.dt.int32
AX = mybir.AxisListType.X
OP = mybir.AluOpType
ACT = mybir.ActivationFunctionType

N = 8192
C = 128
M = 8          # cores
R = N // M     # 1024 rows per core
JC = R // 128  # 8 row-chunks per core
QC = N // 128  # 64 node chunks
CGW = 2048     # column-group width for streaming A
NCG = N // CGW # 4 column groups
EPS = 1e-15

_cache: dict = {}


def _trunc13(x):
    xi = np.ascontiguousarray(x, np.float32).view(np.uint32)
    return (xi & np.uint32(0xFFFFF800)).view(np.float32)


def _build():
    nc = bacc.Bacc("TRN2", target_bir_lowering=False, debug=False, num_devices=M)

    # ---- I/O ----
    a_in = nc.dram_tensor("a", [R, N], F32, kind="ExternalInput")          # per-core row shard
    xa5_in = nc.dram_tensor("xa5", [128, JC * 5], F32, kind="ExternalInput")  # swizzled [x_hi|x_lo|1]
    nodesT_in = nc.dram_tensor("nodesT", [3, N], F32, kind="ExternalInput")
    w2cat_in = nc.dram_tensor("w2cat", [5, C], F32, kind="ExternalInput")  # [w2l; w2r; b2]
    w1cat_in = nc.dram_tensor("w1cat", [5, 2], F32, kind="ExternalInput")  # [w1l; w1r; b1]
    w3cat_in = nc.dram_tensor("w3cat", [5, 1], F32, kind="ExternalInput")  # [w3l; w3r; b3]
    ident_in = nc.dram_tensor("ident", [128, 128], F32, kind="ExternalInput")
    iota_in = nc.dram_tensor("iota", [128, 128], F32, kind="ExternalInput")

    x3_out = nc.dram_tensor("x3", [C, 1], F32, kind="ExternalOutput")
    am_out = nc.dram_tensor("am", [C, 1], I32, kind="ExternalOutput")
    link_out = nc.dram_tensor("link", [1, 1], F32, kind="ExternalOutput")
    ent_out = nc.dram_tensor("ent", [1, 1], F32, kind="ExternalOutput")
    xout_out = nc.dram_tensor("xout", [C, 2], F32, kind="ExternalOutput")

    with tile.TileContext(nc) as tc:
        with (
            tc.tile_pool(name="dram", bufs=1, space="DRAM") as dram,
            tc.tile_pool(name="persist", bufs=1) as pers,
            tc.tile_pool(name="atiles", bufs=4) as pa,
        ):
            # DRAM bounce buffers
            t_in_d = dram.tile([5, N], F32)
            t_out_d = dram.tile([5, N], F32)
            sx_d = dram.tile([N, 130], F32)       # [s | x1] per node row
            ar2_in_d = dram.tile([128, 259], F32)
            ar2_out_d = dram.tile([128, 259], F32)
            rb_d = dram.tile([2, N], F32)

            # persistent SBUF
            s_full = pers.tile([128, QC * 128], F32)     # s, chunk q at [:, q*128:(q+1)*128]
            s_loc = pers.tile([128, JC, 130], F32)        # gathered local [s | x1]
            s_hi = pers.tile([128, JC, 128], F32R)        # fp22-rounded local s
            s_lo = pers.tile([128, JC, 128], F32R)        # remainder (fp22-rounded)
            xa5_sb = pers.tile([128, JC * 5], F32R)
            w2cat = pers.tile([5, C], F32)
            w1cat = pers.tile([5, 2], F32)
            w3cat = pers.tile([5, 1], F32)
            ident = pers.tile([128, 128], F32)
            iota = pers.tile([128, 128], F32)
            ones_col = pers.tile([128, 1], F32)
            eps_col = pers.tile([128, 1], F32)
            zero1 = pers.tile([1, 1], F32)
            sc_nnz = pers.tile([1, 1], F32)
            ent8 = pers.tile([128, JC], F32)
            ar2_sb = pers.tile([128, 259], F32)
            fin_sb = pers.tile([128, 259], F32)

            nc.sync.dma_start(out=xa5_sb, in_=xa5_in.ap().bitcast(F32R))
            nc.sync.dma_start(out=w2cat, in_=w2cat_in.ap())
            nc.sync.dma_start(out=w1cat, in_=w1cat_in.ap())
            nc.sync.dma_start(out=w3cat, in_=w3cat_in.ap())
            nc.sync.dma_start(out=ident, in_=ident_in.ap())
            nc.sync.dma_start(out=iota, in_=iota_in.ap())
            nc.vector.memset(ones_col, 1.0)
            nc.vector.memset(eps_col, EPS)
            nc.vector.memset(zero1, 0.0)

            # ================= phase 1: t = [x_hi|x_lo|1]^T A =================
            with (
                tc.tile_pool(name="p1ps", bufs=2, space="PSUM") as p1ps,
                tc.tile_pool(name="p1sb", bufs=2) as p1sb,
            ):
                for cg in range(NCG):
                    psum_t = p1ps.tile([5, CGW], F32)
                    for j in range(JC):
                        at = pa.tile([128, CGW], F32R, tag="at")
                        nc.sync.dma_start(
                            out=at,
                            in_=a_in.ap()[j * 128:(j + 1) * 128, cg * CGW:(cg + 1) * CGW].bitcast(F32R),
                        )
                        lhs = xa5_sb[:, j * 5:(j + 1) * 5]
                        for i in range(CGW // 512):
                            nc.tensor.matmul(
                                psum_t[:, i * 512:(i + 1) * 512],
                                lhs,
                                at[:, i * 512:(i + 1) * 512],
                                start=(j == 0),
                                stop=(j == JC - 1),
                            )
                    tsg = p1sb.tile([5, CGW], F32)
                    nc.vector.tensor_copy(tsg, psum_t)
                    nc.sync.dma_start(
                        out=t_in_d[:, cg * CGW:(cg + 1) * CGW], in_=tsg
                    )

            nc.gpsimd.collective_compute(
                "AllReduce",
                OP.add,
                ins=[t_in_d.opt()],
                outs=[t_out_d.opt()],
                replica_groups=[list(range(M))],
            )

            # ================= prep: agg, logits, softmax, x1 =================
            with (
                tc.tile_pool(name="prsb", bufs=1) as prsb,
                tc.tile_pool(name="prw", bufs=3) as prw,
                tc.tile_pool(name="prps", bufs=3, space="PSUM") as prps,
            ):
                # reshape the five t rows to [64, 128] tiles (64 lanes, partition-legal)
                t64 = prsb.tile([64, 5, 128], F32)
                for m in range(5):
                    nc.sync.dma_start(
                        out=t64[:, m, :],
                        in_=t_out_d[m:m + 1, :].rearrange("a (q i) -> (a q) i", i=128),
                    )
                deg64 = t64[:, 4, :]
                # nnz(A) = sum(deg)
                deg64r = prsb.tile([64, 1], F32)
                nc.vector.reduce_sum(deg64r, deg64, axis=AX)
                ps_nnz = prps.tile([1, 1], F32, tag="pnnz", bufs=1)
                nc.tensor.matmul(ps_nnz, ones_col[0:64, :], deg64r, start=True, stop=True)
                nc.vector.tensor_copy(sc_nnz, ps_nnz)

                # agg rows: (t_hi + t_lo) / max(deg,1), all in [64,128] layout
                recip64 = prsb.tile([64, 128], F32)
                nc.vector.tensor_scalar_max(recip64, deg64, 1.0)
                nc.vector.reciprocal(recip64, recip64)
                agg64 = prsb.tile([64, 2, 128], F32)
                for m in range(2):
                    nc.vector.tensor_tensor(
                        out=agg64[:, m, :], in0=t64[:, m, :], in1=t64[:, m + 2, :],
                        op=OP.add,
                    )
                    nc.vector.tensor_tensor(
                        out=agg64[:, m, :], in0=agg64[:, m, :], in1=recip64,
                        op=OP.mult,
                    )
                    nc.sync.dma_start(
                        out=rb_d[m:m + 1, :].rearrange("a (q i) -> (a q) i", i=128),
                        in_=agg64[:, m, :],
                    )

                # f5T rows: [agg (2); nodesT+ones (3)] — assembled by DMA only
                f5T = prsb.tile([5, N], F32)
                nc.sync.dma_start(out=f5T[0:2, :], in_=rb_d)
                nc.sync.dma_start(out=f5T[2:5, :], in_=nodesT_in.ap())

                # per-chunk: logits -> softmax -> s ; x1
                for q in range(QC):
                    lhs = f5T[:, q * 128:(q + 1) * 128]
                    ps_lg = prps.tile([128, C], F32, tag="plg", bufs=2)
                    nc.tensor.matmul(ps_lg, lhs, w2cat, start=True, stop=True)
                    ps_x1 = prps.tile([128, 2], F32, tag="px1", bufs=2)
                    nc.tensor.matmul(ps_x1, lhs, w1cat, start=True, stop=True)

                    negmx = prw.tile([128, 1], F32, tag="negmx")
                    nc.vector.reduce_max(negmx, ps_lg, axis=AX, negate=True)
                    ex = prw.tile([128, C], F32, tag="ex")
                    sumx = prw.tile([128, 1], F32, tag="sumx")
                    nc.scalar.activation(
                        ex, ps_lg, ACT.Exp, bias=negmx, scale=1.0, accum_out=sumx
                    )
                    nc.vector.reciprocal(sumx, sumx)
                    s_sl = s_full[:, q * 128:(q + 1) * 128]
                    nc.vector.tensor_scalar_mul(s_sl, ex, sumx)
                    nc.sync.dma_start(
                        out=sx_d[q * 128:(q + 1) * 128, 0:128], in_=s_sl
                    )
                    x1sb = prw.tile([128, 2], F32, tag="x1sb")
                    nc.vector.tensor_copy(x1sb, ps_x1)
                    nc.sync.dma_start(
                        out=sx_d[q * 128:(q + 1) * 128, 128:130], in_=x1sb
                    )

            # ========== gather local rows (partition-id dynamic offset) ==========
            pid = nc.gpsimd.partition_id()
            nc.gpsimd.dma_start(
                out=s_loc,
                in_=sx_d[bass.ds(pid * R, R), :].rearrange("(j p) k -> p j k", p=128),
            )
            # splits: s_hi = 13-bit truncation (exact under f32r), s_lo = remainder
            s_view = s_loc[:, :, 0:128]
            nc.vector.tensor_copy(s_hi, s_view)   # rounds to fp22 on write
            nc.vector.tensor_tensor(out=s_lo, in0=s_view, in1=s_hi, op=OP.subtract)

            # ============ G, x_out, ent partials (local rows) ============
            with tc.tile_pool(name="gxps", bufs=1, space="PSUM") as gxps:
                ps_G = gxps.tile([C, C], F32, tag="psG")
                ps_xo = gxps.tile([C, 2], F32, tag="psxo")
                for j in range(JC):
                    s_j = s_loc[:, j, 0:128]
                    nc.tensor.matmul(
                        ps_G, s_j, s_j, start=(j == 0), stop=(j == JC - 1)
                    )
                    nc.tensor.matmul(
                        ps_xo, s_j, s_loc[:, j, 128:130],
                        start=(j == 0), stop=(j == JC - 1),
                    )
                    ln = pers.tile([128, C], F32, tag="lnent", bufs=2)
                    nc.scalar.activation(ln, s_j, ACT.Ln, bias=eps_col, scale=1.0)
                    nc.vector.tensor_tensor(out=ln, in0=ln, in1=s_j, op=OP.mult)
                    nc.vector.reduce_sum(ent8[:, j:j + 1], ln, axis=AX)
                nc.vector.tensor_copy(ar2_sb[:, 128:256], ps_G)
                nc.vector.tensor_copy(ar2_sb[:, 256:258], ps_xo)
                entrow = pers.tile([128, 1], F32)
                nc.vector.reduce_sum(entrow, ent8, axis=AX)
                ps_ent = gxps.tile([1, 1], F32, tag="psent")
                nc.tensor.matmul(ps_ent, ones_col, entrow, start=True, stop=True)
                nc.vector.memset(ar2_sb[:, 258:259], 0.0)
                nc.vector.tensor_copy(ar2_sb[0:1, 258:259], ps_ent)

            # ================= phase 2: zT = s_loc^T A ; adj_pool =================
            with (
                tc.tile_pool(name="p2ps", bufs=1, space="PSUM") as p2ps,
                tc.tile_pool(name="p2sb", bufs=2) as p2sb,
            ):
                ps_adj = p2ps.tile([C, C], F32, tag="psadj")
                for cg in range(NCG):
                    psum_z = p2ps.tile([128, CGW], F32, tag="psz")
                    for j in range(JC):
                        at = pa.tile([128, CGW], F32R, tag="at")
                        nc.sync.dma_start(
                            out=at,
                            in_=a_in.ap()[j * 128:(j + 1) * 128, cg * CGW:(cg + 1) * CGW].bitcast(F32R),
                        )
                        lh_hi = s_hi[:, j, :]
                        lh_lo = s_lo[:, j, :]
                        for i in range(CGW // 512):
                            rhs = at[:, i * 512:(i + 1) * 512]
                            nc.tensor.matmul(
                                psum_z[:, i * 512:(i + 1) * 512], lh_hi, rhs,
                                start=(j == 0), stop=False,
                            )
                            nc.tensor.matmul(
                                psum_z[:, i * 512:(i + 1) * 512], lh_lo, rhs,
                                start=False, stop=(j == JC - 1),
                            )
                    zt_sb = p2sb.tile([128, CGW], F32, tag="ztsb")
                    nc.vector.tensor_copy(zt_sb, psum_z)
                    for i in range(CGW // 128):
                        q = cg * (CGW // 128) + i
                        ps_tr = p2ps.tile([128, 128], F32, tag="pstr", bufs=2)
                        nc.tensor.transpose(
                            ps_tr, zt_sb[:, i * 128:(i + 1) * 128], ident
                        )
                        z_sb = p2sb.tile([128, 128], F32, tag="zsb", bufs=3)
                        nc.vector.tensor_copy(z_sb, ps_tr)
                        nc.tensor.matmul(
                            ps_adj, z_sb, s_full[:, q * 128:(q + 1) * 128],
                            start=(q == 0), stop=(q == QC - 1),
                        )
                nc.vector.tensor_copy(ar2_sb[:, 0:128], ps_adj)

            nc.sync.dma_start(out=ar2_in_d.opt(), in_=ar2_sb)
            nc.gpsimd.collective_compute(
                "AllReduce",
                OP.add,
                ins=[ar2_in_d.opt()],
                outs=[ar2_out_d.opt()],
                replica_groups=[list(range(M))],
            )
            nc.sync.dma_start(out=fin_sb, in_=ar2_out_d.opt())

            # ===================== epilogue (replicated) =====================
            with (
                tc.tile_pool(name="epsb", bufs=1) as ep,
                tc.tile_pool(name="epps", bufs=1, space="PSUM") as epps,
            ):
                adj = fin_sb[:, 0:128]
                Gm = fin_sb[:, 128:256]
                xo = fin_sb[:, 256:258]
                entc = fin_sb[0:1, 258:259]

                # ent_loss = -sum / N
                ent_sb = ep.tile([1, 1], F32)
                nc.vector.tensor_scalar_mul(ent_sb, entc, -1.0 / N)
                nc.sync.dma_start(out=ent_out.ap(), in_=ent_sb)

                # x_out output
                nc.sync.dma_start(out=xout_out.ap(), in_=xo)

                # link loss: sqrt(nnz - 2 tr(adj) + sum(G^2)) / N^2
                trg = ep.tile([128, 2], F32)
                tmp = ep.tile([128, 128], F32, tag="eptmp")
                nc.vector.tensor_tensor(out=tmp, in0=adj, in1=ident, op=OP.mult)
                nc.vector.reduce_sum(trg[:, 0:1], tmp, axis=AX)
                nc.vector.tensor_tensor(out=tmp, in0=Gm, in1=Gm, op=OP.mult)
                nc.vector.reduce_sum(trg[:, 1:2], tmp, axis=AX)
                ps_tg = epps.tile([1, 2], F32, tag="pstg")
                nc.tensor.matmul(ps_tg, ones_col, trg, start=True, stop=True)
                l2 = ep.tile([1, 1], F32)
                nc.vector.tensor_scalar(
                    out=l2, in0=ps_tg[0:1, 0:1], scalar1=-2.0, scalar2=None, op0=OP.mult
                )
                nc.vector.tensor_tensor(out=l2, in0=l2, in1=ps_tg[0:1, 1:2], op=OP.add)
                nc.vector.tensor_tensor(out=l2, in0=l2, in1=sc_nnz, op=OP.add)
                nc.scalar.activation(l2, l2, ACT.Sqrt, bias=zero1, scale=1.0)
                nc.vector.tensor_scalar_mul(l2, l2, 1.0 / (float(N) * float(N)))
                nc.sync.dma_start(out=link_out.ap(), in_=l2)

                # E = (adj == rowmax); argmax = min index of max
                rmax = ep.tile([128, 1], F32)
                nc.vector.reduce_max(rmax, adj, axis=AX)
                E_sb = ep.tile([128, 128], F32)
                nc.vector.tensor_scalar(
                    out=E_sb, in0=adj, scalar1=rmax, scalar2=None, op0=OP.is_equal
                )
                ta = ep.tile([128, 128], F32, tag="epta")
                nc.vector.tensor_tensor(
                    out=ta, in0=E_sb, in1=iota, op=OP.mult
                )
                tb = ep.tile([128, 128], F32, tag="eptb")
                nc.vector.tensor_scalar(
                    out=tb, in0=E_sb, scalar1=999.0, scalar2=999.0,
                    op0=OP.mult, op1=OP.subtract,
                )
                nc.vector.tensor_tensor(out=ta, in0=ta, in1=tb, op=OP.subtract)
                amf = ep.tile([128, 1], F32)
                nc.vector.tensor_reduce(amf, ta, axis=AX, op=OP.min)
                ami = ep.tile([128, 1], I32)
                nc.vector.tensor_copy(ami, amf)
                nc.sync.dma_start(out=am_out.ap(), in_=ami)

                # x3 = SAGE(x_out, E): aggE = (E^T x_out)/max(degE,1)
                ps_degE = epps.tile([1, 128], F32, tag="psdegE")
                nc.tensor.matmul(ps_degE, ones_col, E_sb, start=True, stop=True)
                degE = ep.tile([1, 128], F32)
                nc.vector.tensor_scalar_max(degE, ps_degE, 1.0)
                nc.vector.reciprocal(degE, degE)
                ps_rEc = epps.tile([128, 1], F32, tag="psrEc")
                nc.tensor.transpose(ps_rEc, degE, ident[0:1, 0:1])
                rEc = ep.tile([128, 1], F32)
                nc.vector.tensor_copy(rEc, ps_rEc)
                ps_EtX = epps.tile([128, 2], F32, tag="psEtX")
                nc.tensor.matmul(ps_EtX, E_sb, xo, start=True, stop=True)
                f3 = ep.tile([128, 5], F32)
                nc.vector.tensor_scalar_mul(f3[:, 0:2], ps_EtX, rEc)
                nc.vector.tensor_copy(f3[:, 2:4], xo)
                nc.vector.memset(f3[:, 4:5], 1.0)
                ps_f3T = epps.tile([5, 128], F32, tag="psf3T")
                nc.tensor.transpose(ps_f3T, f3, ident)
                f3T = ep.tile([5, 128], F32)
                nc.vector.tensor_copy(f3T, ps_f3T)
                ps_x3 = epps.tile([128, 1], F32, tag="psx3")
                nc.tensor.matmul(ps_x3, f3T, w3cat, start=True, stop=True)
                x3_sb = ep.tile([128, 1], F32)
                nc.vector.tensor_copy(x3_sb, ps_x3)
                nc.sync.dma_start(out=x3_out.ap(), in_=x3_sb)

    nc.compile()
    return nc


def kernel(nodes, adjs, w1l, b1, w1r, w2l, b2, w2r, w3l, b3, w3r):
    nodes = np.ascontiguousarray(np.asarray(nodes, np.float32))
    adjs = np.ascontiguousarray(np.asarray(adjs, np.float32))

    if "nc" not in _cache:
        _cache["nc"] = _build()
    nc = _cache["nc"]

    x_hi = _trunc13(nodes)
    x_lo = (nodes - x_hi).astype(np.float32)
    w2cat = np.concatenate(
        [np.asarray(w2l, np.float32), np.asarray(w2r, np.float32),
         np.asarray(b2, np.float32).reshape(1, C)], axis=0
    )
    w1cat = np.concatenate(
        [np.asarray(w1l, np.float32), np.asarray(w1r, np.float32),
         np.asarray(b1, np.float32).reshape(1, 2)], axis=0
    )
    w3cat = np.concatenate(
        [np.asarray(w3l, np.float32), np.asarray(w3r, np.float32),
         np.asarray(b3, np.float32).reshape(1, 1)], axis=0
    )
    nodesT = np.ascontiguousarray(
        np.concatenate([nodes.T, np.ones((1, N), np.float32)], axis=0)
    )
    ident = np.eye(128, dtype=np.float32)
    iota = np.tile(np.arange(128, dtype=np.float32), (128, 1))

    in_maps = []
    for c in range(M):
        sl = slice(c * R, (c + 1) * R)
        xa5 = np.concatenate(
            [x_hi[sl], x_lo[sl], np.ones((R, 1), np.float32)], axis=1
        )  # [R, 5]
        xa5sw = np.ascontiguousarray(
            xa5.reshape(JC, 128, 5).transpose(1, 0, 2).reshape(128, JC * 5)
        )
        in_maps.append(
            dict(
                a=np.ascontiguousarray(adjs[sl]),
                xa5=xa5sw,
                nodesT=nodesT,
                w2cat=w2cat,
                w1cat=w1cat,
                w3cat=w3cat,
                ident=ident,
                iota=iota,
            )
        )

    res = bass_utils.run_bass_kernel_spmd(nc, in_maps, core_ids=list(range(M)))
    r0 = res.results[0]

    x3 = r0["x3"].reshape(C).astype(np.float32)
    am = r0["am"].reshape(C).astype(np.int32)
    edge_index = np.stack([np.arange(C, dtype=np.int32), am]).astype(np.int32)
    link_loss = np.float32(r0["link"].reshape(()))
    ent_loss = np.float32(r0["ent"].reshape(()))
    x_out = r0["xout"].reshape(C, 2).astype(np.float32)
    return x3, edge_index, link_loss, ent_loss, x_out


# revision 24
# speedup vs baseline: 1.0424x; 1.0424x over previous
"""Bass/Tile TRN2 kernel for nn_Bonv_89369679495333 (SAGE-conv + dense_diff_pool).

Strategy (8 NeuronCores, row-sharded over the N=8192 nodes, 1024 rows/core):
  phase 1: t = [x_hi | x_lo | 1]^T @ A_shard   (f32r split matmuls, exact to ~2^-26)
           -> AllReduce [5, 8192]  -> t_x = Atx rows, deg = colsum row
  prep:    agg = t_x / max(deg,1); logits = [agg;nodes;1]^T W2cat ; s = softmax
           x1 = [agg;nodes;1]^T W1cat   (conv1 output)
           s, x1 written to DRAM; each core gathers its local 1024 rows via a
           partition_id-offset DMA (the program is SPMD-identical on all cores).
  phase 2: zT_c = s_local^T @ A_shard  (two f32r passes: s_hi 13-bit + remainder)
           z chunks PE-transposed; adj_pool partial = z^T s accumulated in PSUM
           G = s_loc^T s_loc, x_out = s_loc^T x1_loc, ent partial
           -> AllReduce [128, 259]
  epilogue (replicated): E = rowmax-equality, argmax, x3 = SAGE(E), link_loss via
           ||A - s s^T||^2 = nnz(A) - 2 tr(adj_pool) + ||s^T s||^2.
"""

import numpy as np

import concourse.bass as bass
import concourse.bacc as bacc
import concourse.tile as tile
import concourse.mybir as mybir
import concourse.bass_utils as bass_utils

F32 = mybir.dt.float32
F32R = mybir.dt.float32r
BF16 = mybir.dt.bfloat16
I32 = mybir.dt.int32
AX = mybir.AxisListType.X
OP = mybir.AluOpType
ACT = mybir.ActivationFunctionType

N = 8192
C = 128
M = 8          # cores
R = N // M     # 1024 rows per core
JC = R // 128  # 8 row-chunks per core
QC = N // 128  # 64 node chunks
CGW = 2048     # column-group width for streaming A
NCG = N // CGW # 4 column groups
EPS = 1e-15

_cache: dict = {}


class _StopBuild(Exception):
    pass


def _trunc13(x):
    xi = np.ascontiguousarray(x, np.float32).view(np.uint32)
    return (xi & np.uint32(0xFFFFF800)).view(np.float32)


def _build(sim=False, upto=99):
    nc = bacc.Bacc("TRN2", target_bir_lowering=False, debug=False, num_devices=M)

    # ---- I/O ----
    a_in = nc.dram_tensor("a", [R, N], F32, kind="ExternalInput")          # per-core row shard
    xa5_in = nc.dram_tensor("xa5", [128, JC * 5], F32, kind="ExternalInput")  # swizzled [x_hi|x_lo|1]
    nodesT_in = nc.dram_tensor("nodesT", [3, N], F32, kind="ExternalInput")
    w2cat_in = nc.dram_tensor("w2cat", [5, C], F32, kind="ExternalInput")  # [w2l; w2r; b2]
    w1cat_in = nc.dram_tensor("w1cat", [5, 2], F32, kind="ExternalInput")  # [w1l; w1r; b1]
    w3cat_in = nc.dram_tensor("w3cat", [5, 1], F32, kind="ExternalInput")  # [w3l; w3r; b3]
    ident_in = nc.dram_tensor("ident", [128, 128], F32, kind="ExternalInput")
    iota_in = nc.dram_tensor("iota", [128, 128], F32, kind="ExternalInput")

    outp_out = nc.dram_tensor("outp", [4, 130], F32, kind="ExternalOutput")

    try:
      with tile.TileContext(nc) as tc:
        with (
            tc.tile_pool(name="dram", bufs=1, space="DRAM") as dram,
            tc.tile_pool(name="persist", bufs=1) as pers,
            tc.tile_pool(name="atiles", bufs=10) as pa,
        ):
            # DRAM bounce buffers
            t_in_d = dram.tile([5, N], F32)
            t_out_d = dram.tile([5, N], F32)
            sx_d = dram.tile([128, QC, 130], F32)  # [s | x1], p-major chunk layout
            ar2_in_d = dram.tile([128, 259], F32)
            ar2_out_d = dram.tile([128, 259], F32)
            rb_d = dram.tile([2, N], F32)

            # persistent SBUF
            s_full = pers.tile([128, QC * 128], F32)     # s, chunk q at [:, q*128:(q+1)*128]
            s_loc = pers.tile([128, JC, 130], F32)        # gathered local [s | x1]
            s_hi = pers.tile([128, JC, 128], F32R)        # fp22-rounded local s
            s_lo = pers.tile([128, JC, 128], F32R)        # remainder (fp22-rounded)
            xa5_sb = pers.tile([128, JC * 5], F32R)
            w2cat = pers.tile([5, C], F32)
            w1cat = pers.tile([5, 2], F32)
            w3cat = pers.tile([5, 1], F32)
            ident = pers.tile([128, 128], F32)
            iota = pers.tile([128, 128], F32)
            ones_col = pers.tile([128, 1], F32)
            eps_col = pers.tile([128, 1], F32)
            zero1 = pers.tile([1, 1], F32)
            zero_col = pers.tile([128, 1], F32)
            sc_nnz = pers.tile([1, 1], F32)
            ent8 = pers.tile([128, JC], F32)
            x1_all = pers.tile([128, QC, 2], F32)
            ar2_sb = pers.tile([128, 259], F32)
            fin_sb = pers.tile([128, 259], F32)

            nc.sync.dma_start(out=xa5_sb, in_=xa5_in.ap().bitcast(F32R))
            nc.sync.dma_start(out=w2cat, in_=w2cat_in.ap())
            nc.sync.dma_start(out=w1cat, in_=w1cat_in.ap())
            nc.sync.dma_start(out=w3cat, in_=w3cat_in.ap())
            nc.sync.dma_start(out=ident, in_=ident_in.ap())
            nc.sync.dma_start(out=iota, in_=iota_in.ap())
            nc.vector.memset(ones_col, 1.0)
            nc.vector.memset(eps_col, EPS)
            nc.vector.memset(zero1, 0.0)
            nc.vector.memset(zero_col, 0.0)

            # ================= phase 1: t = [x_hi|x_lo|1]^T A =================
            with (
                tc.tile_pool(name="p1ps", bufs=2, space="PSUM") as p1ps,
                tc.tile_pool(name="p1sb", bufs=2) as p1sb,
            ):
                for cg in range(NCG):
                    psum_t = p1ps.tile([5, CGW], F32)
                    for j in range(JC):
                        at = pa.tile([128, CGW], F32R, tag="at")
                        nc.sync.dma_start(
                            out=at,
                            in_=a_in.ap()[j * 128:(j + 1) * 128, cg * CGW:(cg + 1) * CGW].bitcast(F32R),
                        )
                        lhs = xa5_sb[:, j * 5:(j + 1) * 5]
                        for i in range(CGW // 512):
                            nc.tensor.matmul(
                                psum_t[:, i * 512:(i + 1) * 512],
                                lhs,
                                at[:, i * 512:(i + 1) * 512],
                                start=(j == 0),
                                stop=(j == JC - 1),
                            )
                    tsg = p1sb.tile([5, CGW], F32)
                    nc.vector.tensor_copy(tsg, psum_t)
                    nc.sync.dma_start(
                        out=t_in_d[:, cg * CGW:(cg + 1) * CGW], in_=tsg
                    )

            if sim:
                nc.scalar.dma_start(out=t_out_d.opt(), in_=t_in_d.opt())
            else:
                nc.gpsimd.collective_compute(
                    "AllReduce",
                    OP.add,
                    ins=[t_in_d.opt()],
                    outs=[t_out_d.opt()],
                    replica_groups=[list(range(M))],
                )

            # ================= prep: agg, logits, softmax, x1 =================
            if upto < 2:
                raise _StopBuild
            with (
                tc.tile_pool(name="prsb", bufs=1) as prsb,
                tc.tile_pool(name="prw", bufs=4) as prw,
                tc.tile_pool(name="prps", bufs=3, space="PSUM") as prps,
            ):
                # reshape the five t rows to [64, 5, 128] in one DMA (64 lanes)
                t64 = prsb.tile([64, 5, 128], F32)
                nc.scalar.dma_start(
                    out=t64,
                    in_=t_out_d.rearrange("m (q i) -> q m i", i=128),
                )
                deg64 = t64[:, 4, :]
                # nnz(A) = sum(deg)
                deg64r = prsb.tile([64, 1], F32)
                nc.vector.reduce_sum(deg64r, deg64, axis=AX)
                ps_nnz = prps.tile([1, 1], F32, tag="pnnz", bufs=1)
                nc.tensor.matmul(ps_nnz, ones_col[0:64, :], deg64r, start=True, stop=True)
                nc.vector.tensor_copy(sc_nnz, ps_nnz)

                # agg rows: (t_hi + t_lo) / max(deg,1), all in [64,128] layout
                recip64 = prsb.tile([64, 128], F32)
                nc.vector.tensor_scalar_max(recip64, deg64, 1.0)
                nc.vector.reciprocal(recip64, recip64)
                agg64 = prsb.tile([64, 2, 128], F32)
                for m in range(2):
                    nc.vector.tensor_tensor(
                        out=agg64[:, m, :], in0=t64[:, m, :], in1=t64[:, m + 2, :],
                        op=OP.add,
                    )
                    nc.vector.tensor_tensor(
                        out=agg64[:, m, :], in0=agg64[:, m, :], in1=recip64,
                        op=OP.mult,
                    )
                nc.scalar.dma_start(
                    out=rb_d.rearrange("m (q i) -> q m i", i=128),
                    in_=agg64,
                )

                # f5T rows: [agg (2); nodesT+ones (3)] — assembled by DMA only
                f5T = prsb.tile([5, N], F32)
                nc.scalar.dma_start(out=f5T[0:2, :], in_=rb_d)
                nc.scalar.dma_start(out=f5T[2:5, :], in_=nodesT_in.ap())

                # per-chunk: logits -> softmax -> s ; x1
                for q in range(QC):
                    lhs = f5T[:, q * 128:(q + 1) * 128]
                    ps_lg = prps.tile([128, C], F32, tag="plg", bufs=3)
                    nc.tensor.matmul(ps_lg, lhs, w2cat, start=True, stop=True)
                    ps_x1 = prps.tile([128, 2], F32, tag="px1", bufs=3)
                    nc.tensor.matmul(ps_x1, lhs, w1cat, start=True, stop=True)

                    negmx = prw.tile([128, 1], F32, tag="negmx")
                    nc.vector.reduce_max(negmx, ps_lg, axis=AX, negate=True)
                    ex = prw.tile([128, C], F32, tag="ex")
                    sumx = prw.tile([128, 1], F32, tag="sumx")
                    nc.scalar.activation(
                        ex, ps_lg, ACT.Exp, bias=negmx, scale=1.0, accum_out=sumx
                    )
                    nc.vector.reciprocal(sumx, sumx)
                    s_sl = s_full[:, q * 128:(q + 1) * 128]
                    nc.vector.tensor_scalar_mul(s_sl, ex, sumx)
                    nc.vector.tensor_copy(x1_all[:, q, :], ps_x1)

            # ========== gather local rows (partition-id dynamic offset) ==========
            if upto < 3:
                raise _StopBuild
            nc.scalar.dma_start(
                out=sx_d[:, :, 0:128],
                in_=s_full.rearrange("p (q k) -> p q k", k=128),
            )
            nc.scalar.dma_start(out=sx_d[:, :, 128:130], in_=x1_all)
            pid = nc.gpsimd.partition_id()
            nc.gpsimd.dma_start(
                out=s_loc,
                in_=sx_d[:, bass.ds(pid * JC, JC), :],
            )
            # splits: s_hi = 13-bit truncation (exact under f32r), s_lo = remainder
            s_view = s_loc[:, :, 0:128]
            nc.vector.tensor_copy(s_hi, s_view)   # rounds to fp22 on write
            nc.vector.tensor_tensor(out=s_lo, in0=s_view, in1=s_hi, op=OP.subtract)

            # ============ G, x_out, ent partials (local rows) ============
            with tc.tile_pool(name="gxps", bufs=1, space="PSUM") as gxps:
                ps_G = gxps.tile([C, C], F32, tag="psG")
                ps_xo = gxps.tile([C, 2], F32, tag="psxo")
                for j in range(JC):
                    s_j = s_loc[:, j, 0:128]
                    nc.tensor.matmul(
                        ps_G, s_j, s_j, start=(j == 0), stop=(j == JC - 1)
                    )
                    nc.tensor.matmul(
                        ps_xo, s_j, s_loc[:, j, 128:130],
                        start=(j == 0), stop=(j == JC - 1),
                    )
                    ln = pers.tile([128, C], F32, tag="lnent", bufs=2)
                    nc.scalar.activation(ln, s_j, ACT.Ln, bias=eps_col, scale=1.0)
                    nc.vector.tensor_tensor(out=ln, in0=ln, in1=s_j, op=OP.mult)
                    nc.vector.reduce_sum(ent8[:, j:j + 1], ln, axis=AX)
                nc.vector.tensor_copy(ar2_sb[:, 128:256], ps_G)
                nc.vector.tensor_copy(ar2_sb[:, 256:258], ps_xo)
                entrow = pers.tile([128, 1], F32)
                nc.vector.reduce_sum(entrow, ent8, axis=AX)
                ps_ent = gxps.tile([1, 1], F32, tag="psent")
                nc.tensor.matmul(ps_ent, ones_col, entrow, start=True, stop=True)
                nc.vector.memset(ar2_sb[:, 258:259], 0.0)
                nc.vector.tensor_copy(ar2_sb[0:1, 258:259], ps_ent)

            # ================= phase 2: zT = s_loc^T A ; adj_pool =================
            if upto < 4:
                raise _StopBuild
            with (
                tc.tile_pool(name="p2ps", bufs=1, space="PSUM") as p2ps,
                tc.tile_pool(name="p2sb", bufs=2) as p2sb,
            ):
                ps_adj = p2ps.tile([C, C], F32, tag="psadj")
                for cg in range(NCG):
                    psum_z = p2ps.tile([128, CGW], F32, tag="psz")
                    for j in range(JC):
                        at = pa.tile([128, CGW], F32R, tag="at")
                        nc.sync.dma_start(
                            out=at,
                            in_=a_in.ap()[j * 128:(j + 1) * 128, cg * CGW:(cg + 1) * CGW].bitcast(F32R),
                        )
                        lh_hi = s_hi[:, j, :]
                        lh_lo = s_lo[:, j, :]
                        for i in range(CGW // 512):
                            rhs = at[:, i * 512:(i + 1) * 512]
                            nc.tensor.matmul(
                                psum_z[:, i * 512:(i + 1) * 512], lh_hi, rhs,
                                start=(j == 0), stop=False,
                            )
                            nc.tensor.matmul(
                                psum_z[:, i * 512:(i + 1) * 512], lh_lo, rhs,
                                start=False, stop=(j == JC - 1),
                            )
                    zt_sb = p2sb.tile([128, CGW], F32, tag="ztsb")
                    nc.vector.tensor_copy(zt_sb, psum_z)
                    for i in range(CGW // 128):
                        q = cg * (CGW // 128) + i
                        ps_tr = p2ps.tile([128, 128], F32, tag="pstr", bufs=2)
                        nc.tensor.transpose(
                            ps_tr, zt_sb[:, i * 128:(i + 1) * 128], ident
                        )
                        z_sb = p2sb.tile([128, 128], F32, tag="zsb", bufs=3)
                        nc.vector.tensor_copy(z_sb, ps_tr)
                        nc.tensor.matmul(
                            ps_adj, z_sb, s_full[:, q * 128:(q + 1) * 128],
                            start=(q == 0), stop=(q == QC - 1),
                        )
                nc.vector.tensor_copy(ar2_sb[:, 0:128], ps_adj)

            nc.scalar.dma_start(out=ar2_in_d.opt(), in_=ar2_sb)
            if sim:
                nc.scalar.dma_start(out=ar2_out_d.opt(), in_=ar2_in_d.opt())
            else:
                nc.gpsimd.collective_compute(
                    "AllReduce",
                    OP.add,
                    ins=[ar2_in_d.opt()],
                    outs=[ar2_out_d.opt()],
                    replica_groups=[list(range(M))],
                )
            nc.scalar.dma_start(out=fin_sb, in_=ar2_out_d.opt())

            # ===================== epilogue (replicated) =====================
            if upto < 5:
                raise _StopBuild
            with (
                tc.tile_pool(name="epsb", bufs=1) as ep,
                tc.tile_pool(name="epps", bufs=1, space="PSUM") as epps,
            ):
                adj = fin_sb[:, 0:128]
                Gm = fin_sb[:, 128:256]
                xo = fin_sb[:, 256:258]
                entc = fin_sb[0:1, 258:259]

                # ent_loss = -sum / N
                ent_sb = ep.tile([1, 1], F32)
                nc.vector.tensor_scalar_mul(ent_sb, entc, -1.0 / N)

                if upto < 6:
                    raise _StopBuild
                # link loss: sqrt(nnz - 2 tr(adj) + sum(G^2)) / N^2
                trg = ep.tile([128, 2], F32)
                tmp = ep.tile([128, 128], F32, tag="eptmp")
                nc.vector.tensor_tensor(out=tmp, in0=adj, in1=ident, op=OP.mult)
                nc.vector.reduce_sum(trg[:, 0:1], tmp, axis=AX)
                nc.vector.tensor_tensor(out=tmp, in0=Gm, in1=Gm, op=OP.mult)
                nc.vector.reduce_sum(trg[:, 1:2], tmp, axis=AX)
                ps_tg = epps.tile([1, 2], F32, tag="pstg")
                nc.tensor.matmul(ps_tg, ones_col, trg, start=True, stop=True)
                l2 = ep.tile([1, 1], F32)
                nc.vector.tensor_scalar(
                    out=l2, in0=ps_tg[0:1, 0:1], scalar1=-2.0, scalar2=None, op0=OP.mult
                )
                nc.vector.tensor_tensor(out=l2, in0=l2, in1=ps_tg[0:1, 1:2], op=OP.add)
                nc.vector.tensor_tensor(out=l2, in0=l2, in1=sc_nnz, op=OP.add)
                nc.scalar.activation(l2, l2, ACT.Sqrt, bias=zero1, scale=1.0)
                nc.vector.tensor_scalar_mul(l2, l2, 1.0 / (float(N) * float(N)))

                if upto < 7:
                    raise _StopBuild
                # E = (adj == rowmax); argmax = min index of max
                rmax = ep.tile([128, 1], F32)
                nc.vector.reduce_max(rmax, adj, axis=AX)
                E_sb = ep.tile([128, 128], F32)
                nc.vector.tensor_scalar(
                    out=E_sb, in0=adj, scalar1=rmax, scalar2=None, op0=OP.is_equal
                )
                if upto < 7.2:
                    raise _StopBuild
                ta = ep.tile([128, 128], F32, tag="epta")
                nc.vector.tensor_tensor(
                    out=ta, in0=E_sb, in1=iota, op=OP.mult
                )
                tb = ep.tile([128, 128], F32, tag="eptb")
                nc.vector.tensor_scalar(
                    out=tb, in0=E_sb, scalar1=999.0, scalar2=999.0,
                    op0=OP.mult, op1=OP.subtract,
                )
                nc.vector.tensor_tensor(out=ta, in0=ta, in1=tb, op=OP.subtract)
                if upto < 7.4:
                    raise _StopBuild
                amf = ep.tile([128, 1], F32)
                nc.vector.tensor_reduce(amf, ta, axis=AX, op=OP.min)

                if upto < 8:
                    raise _StopBuild
                # x3 = SAGE(x_out, E): aggE = (E^T x_out)/max(degE,1)
                ps_degE = epps.tile([1, 128], F32, tag="psdegE")
                nc.tensor.matmul(ps_degE, ones_col, E_sb, start=True, stop=True)
                if upto < 8.1:
                    raise _StopBuild
                degE = ep.tile([1, 128], F32)
                nc.vector.tensor_scalar_max(degE, ps_degE, 1.0)
                nc.vector.reciprocal(degE, degE)
                ps_rEc = epps.tile([128, 1], F32, tag="psrEc")
                nc.tensor.transpose(ps_rEc, degE, ident[0:1, 0:1])
                rEc = ep.tile([128, 1], F32)
                nc.vector.tensor_copy(rEc, ps_rEc)
                if upto < 8.3:
                    raise _StopBuild
                ps_EtX = epps.tile([128, 2], F32, tag="psEtX")
                nc.tensor.matmul(ps_EtX, E_sb, xo, start=True, stop=True)
                f3 = ep.tile([128, 5], F32)
                nc.vector.tensor_scalar_mul(f3[:, 0:2], ps_EtX, rEc)
                nc.vector.tensor_copy(f3[:, 2:4], xo)
                nc.vector.memset(f3[:, 4:5], 1.0)
                if upto < 8.5:
                    raise _StopBuild
                ps_f3T = epps.tile([5, 128], F32, tag="psf3T")
                nc.tensor.transpose(ps_f3T, f3, ident)
                f3T = ep.tile([5, 128], F32)
                nc.vector.tensor_copy(f3T, ps_f3T)
                if upto < 8.7:
                    raise _StopBuild
                ps_x3 = epps.tile([128, 1], F32, tag="psx3")
                nc.tensor.matmul(ps_x3, f3T, w3cat, start=True, stop=True)
                # pack [x3 | amf | xout] -> transpose -> [4, 130] with scalars
                pk = ep.tile([128, 4], F32)
                nc.vector.tensor_copy(pk[:, 0:1], ps_x3)
                nc.vector.tensor_copy(pk[:, 1:2], amf)
                nc.vector.tensor_copy(pk[:, 2:4], xo)
                ps_pk = epps.tile([4, 128], F32, tag="pspk")
                nc.tensor.transpose(ps_pk, pk, ident)
                out4 = ep.tile([4, 130], F32)
                nc.vector.tensor_copy(out4[:, 0:128], ps_pk)
                nc.vector.memset(out4[:, 128:130], 0.0)
                nc.vector.tensor_copy(out4[0:1, 128:129], l2)
                nc.vector.tensor_copy(out4[0:1, 129:130], ent_sb)
                nc.scalar.dma_start(out=outp_out.ap(), in_=out4)

    except _StopBuild:
        pass
    nc.compile()
    return nc


def kernel(nodes, adjs, w1l, b1, w1r, w2l, b2, w2r, w3l, b3, w3r):
    nodes = np.ascontiguousarray(np.asarray(nodes, np.float32))
    adjs = np.ascontiguousarray(np.asarray(adjs, np.float32))

    if "nc" not in _cache:
        _cache["nc"] = _build()
    nc = _cache["nc"]

    x_hi = _trunc13(nodes)
    x_lo = (nodes - x_hi).astype(np.float32)
    w2cat = np.concatenate(
        [np.asarray(w2l, np.float32), np.asarray(w2r, np.float32),
         np.asarray(b2, np.float32).reshape(1, C)], axis=0
    )
    w1cat = np.concatenate(
        [np.asarray(w1l, np.float32), np.asarray(w1r, np.float32),
         np.asarray(b1, np.float32).reshape(1, 2)], axis=0
    )
    w3cat = np.concatenate(
        [np.asarray(w3l, np.float32), np.asarray(w3r, np.float32),
         np.asarray(b3, np.float32).reshape(1, 1)], axis=0
    )
    nodesT = np.ascontiguousarray(
        np.concatenate([nodes.T, np.ones((1, N), np.float32)], axis=0)
    )
    ident = np.eye(128, dtype=np.float32)
    iota = np.tile(np.arange(128, dtype=np.float32), (128, 1))

    in_maps = []
    for c in range(M):
        sl = slice(c * R, (c + 1) * R)
        xa5 = np.concatenate(
            [x_hi[sl], x_lo[sl], np.ones((R, 1), np.float32)], axis=1
        )  # [R, 5]
        xa5sw = np.ascontiguousarray(
            xa5.reshape(JC, 128, 5).transpose(1, 0, 2).reshape(128, JC * 5)
        )
        in_maps.append(
            dict(
                a=np.ascontiguousarray(adjs[sl]),
                xa5=xa5sw,
                nodesT=nodesT,
                w2cat=w2cat,
                w1cat=w1cat,
                w3cat=w3cat,
                ident=ident,
                iota=iota,
            )
        )

    res = bass_utils.run_bass_kernel_spmd(nc, in_maps, core_ids=list(range(M)))
    r0 = res.results[0]

    outp = np.asarray(r0["outp"], np.float32)  # [4, 130]
    x3 = outp[0, 0:C].astype(np.float32)
    am = np.rint(outp[1, 0:C]).astype(np.int32)
    edge_index = np.stack([np.arange(C, dtype=np.int32), am]).astype(np.int32)
    x_out = np.ascontiguousarray(outp[2:4, 0:C].T, dtype=np.float32)
    link_loss = np.float32(outp[0, 128])
    ent_loss = np.float32(outp[0, 129])
    return x3, edge_index, link_loss, ent_loss, x_out
